# revision 4
# baseline (speedup 1.0000x reference)
"""Trainium2 Bass kernel for nn_Attention_88510686036059 (gnn message passing).

Strategy: data-parallel over batch B=8 -> one NeuronCore per batch element.
Per core (batch b):
  x_b (L=32, A=128, D=256) pre-transposed on host to xT (D, L*A).
  kT/qT = Wk^T/Wq^T projections via PE (W stationary), layout (dout, l*A+a).
  scores per (h,l): S = kT_hl^T @ qT_hl on PE (K=hd=32), exp-evicted by ACT
  into E[i, j, l] SBUF tiles; softmax denom + GAT terms via DVE reductions.
  weight head: algebraic collapse -- out/gat never materialized:
    logits[i,j] = sum_h recip[h,i,j]*G1[h,i,j] + (sum_h recip*G2)^T[i,j] + b
    G1[h,i,j] = sum_l E[h,i,j,l]*vw1[h,i,l],  vw1 = x @ (Wv . Watt-half folding)
  attn = E * recip broadcast, DMA'd per (h): 2MB contiguous chunks.
"""

import numpy as np

L, B, A, D, H = 32, 8, 128, 256, 8
HD = D // H
P = 128
N = L * A  # 4096

LAST_RESULT = None
_NC_CACHE = {}


def _build_nc():
    import concourse.bass as bass  # noqa: F401
    import concourse.mybir as mybir
    import concourse.tile as tile
    from concourse import bacc
    from concourse._compat import axon_active
    from concourse.masks import make_identity

    f32 = mybir.dt.float32
    AF = mybir.ActivationFunctionType
    OP = mybir.AluOpType
    AX = mybir.AxisListType

    nc = bacc.Bacc(
        "TRN2", target_bir_lowering=False, debug=False, num_devices=B
    )

    xT_d = nc.dram_tensor("xT", (2, P, N), f32, kind="ExternalInput")
    wk_d = nc.dram_tensor("wk", (2, P, D), f32, kind="ExternalInput")
    wq_d = nc.dram_tensor("wq", (2, P, D), f32, kind="ExternalInput")
    wv_d = nc.dram_tensor("wv12", (2, P, 2 * H), f32, kind="ExternalInput")
    mask_d = nc.dram_tensor("mask", (P, A), f32, kind="ExternalInput")
    bias_d = nc.dram_tensor("biasv", (P, 1), f32, kind="ExternalInput")
    attn_d = nc.dram_tensor("attn", (H, A, A, L), f32, kind="ExternalOutput")
    weight_d = nc.dram_tensor("weight", (A, A), f32, kind="ExternalOutput")

    with tile.TileContext(nc) as tc:
        with (
            tc.tile_pool(name="const", bufs=1) as cpool,
            tc.tile_pool(name="kqt", bufs=1) as kqt_pool,
            tc.tile_pool(name="small", bufs=1) as small,
            tc.tile_pool(name="smallh", bufs=2) as smallh,
        ):
            # persistent tiles
            kT = [kqt_pool.tile([P, N], f32, tag=f"kT{m}", name=f"kT{m}") for m in range(2)]
            qT = [kqt_pool.tile([P, N], f32, tag=f"qT{m}", name=f"qT{m}") for m in range(2)]
            vw_sb = small.tile([P, 2 * H, L], f32, tag="vw")  # [a, g, l]
            idn = cpool.tile([P, P], f32, tag="idn")
            make_identity(nc, idn[:])
            mask_sb = cpool.tile([P, A], f32, tag="mask")
            bias_sb = cpool.tile([P, 1], f32, tag="bias")
            nc.sync.dma_start(mask_sb[:], mask_d[:])
            nc.sync.dma_start(bias_sb[:], bias_d[:])
            T1 = small.tile([P, A], f32, tag="T1")
            T2 = small.tile([P, A], f32, tag="T2")

            # ---------------- phase 1: projections ----------------
            with (
                tc.tile_pool(name="xpool", bufs=1) as xpool,
                tc.tile_pool(name="psA", bufs=4, space="PSUM") as psA,
                tc.tile_pool(name="psB", bufs=2, space="PSUM") as psB,
            ):
                xT = [xpool.tile([P, N], f32, tag=f"xT{c}", name=f"xT{c}") for c in range(2)]
                for c in range(2):
                    nc.sync.dma_start(xT[c][:], xT_d[c])
                wk_sb = cpool.tile([P, 2, D], f32, tag="wk")
                wq_sb = cpool.tile([P, 2, D], f32, tag="wq")
                wv_sb = cpool.tile([P, 2, 2 * H], f32, tag="wv")
                nc.sync.dma_start(wk_sb[:], wk_d[:].rearrange("c p d -> p c d"))
                nc.sync.dma_start(wq_sb[:], wq_d[:].rearrange("c p d -> p c d"))
                nc.sync.dma_start(wv_sb[:], wv_d[:].rearrange("c p d -> p c d"))

                # kT/qT = W^T x^T : lhsT = W chunk (K=din 128, M=dout 128)
                for wsb, dst in ((wk_sb, kT), (wq_sb, qT)):
                    for m in range(2):
                        for ns in range(8):
                            ps = psA.tile([P, 512], f32, tag="psA", name="psA_t")
                            for c in range(2):
                                nc.tensor.matmul(
                                    ps[:],
                                    wsb[:, c, m * P : (m + 1) * P],
                                    xT[c][:, ns * 512 : (ns + 1) * 512],
                                    start=(c == 0),
                                    stop=(c == 1),
                                )
                            nc.scalar.copy(
                                dst[m][:, ns * 512 : (ns + 1) * 512], ps[:]
                            )

                # vw[a, g, l] = sum_c xT[c, l*128+a] * wv12[c, g]
                for l in range(L):
                    ps = psB.tile([P, 2 * H], f32, tag="psB", name="psB_t")
                    for c in range(2):
                        nc.tensor.matmul(
                            ps[:],
                            xT[c][:, l * P : (l + 1) * P],
                            wv_sb[:, c, :],
                            start=(c == 0),
                            stop=(c == 1),
                        )
                    nc.vector.tensor_copy(vw_sb[:, :, l], ps[:])

            # ---------------- phase 2: scores/softmax/gat ----------------
            with (
                tc.tile_pool(name="epool", bufs=2) as epool,
                tc.tile_pool(name="apool", bufs=2) as apool,
                tc.tile_pool(name="gpool", bufs=2) as gpool,
                tc.tile_pool(name="psS", bufs=4, space="PSUM") as psS,
                tc.tile_pool(name="psT", bufs=1, space="PSUM") as psT,
            ):
                for h in range(H):
                    ch, rh = h // 4, (h % 4) * 32
                    E = epool.tile([P, A, L], f32, tag="E", name="E_t")
                    at = apool.tile([P, A, L], f32, tag="at", name="at_t")
                    for l in range(L):
                        ps = psS.tile([P, A], f32, tag="psS", name="psS_t")
                        nc.tensor.matmul(
                            ps[:],
                            kT[ch][rh : rh + 32, l * P : (l + 1) * P],
                            qT[ch][rh : rh + 32, l * P : (l + 1) * P],
                            start=True,
                            stop=True,
                            tile_position=(rh, 0),
                        )
                        # E = exp(S / sqrt(D))
                        nc.scalar.activation(
                            E[:, :, l], ps[:], AF.Exp, scale=1.0 / 16.0
                        )
                    den = smallh.tile([P, A], f32, tag="den", name="den_t")
                    rec = smallh.tile([P, A], f32, tag="rec", name="rec_t")
                    nc.vector.tensor_reduce(den[:], E[:], axis=AX.X, op=OP.add)
                    nc.vector.reciprocal(rec[:], den[:])

                    # G terms for the gat head
                    for gi, Tacc in ((0, T1), (1, T2)):
                        gt = gpool.tile([P, A, L], f32, tag="gt", name="gt_t")
                        nc.gpsimd.tensor_tensor(
                            gt[:],
                            E[:],
                            vw_sb[:, gi * H + h, None, :].to_broadcast(
                                (P, A, L)
                            ),
                            OP.mult,
                        )
                        g = smallh.tile([P, A], f32, tag=f"g{gi}", name=f"g{gi}_t")
                        nc.vector.tensor_reduce(
                            g[:], gt[:], axis=AX.X, op=OP.add
                        )
                        nc.vector.tensor_tensor(g[:], g[:], rec[:], OP.mult)
                        if h == 0:
                            nc.vector.tensor_copy(Tacc[:], g[:])
                        else:
                            nc.vector.tensor_tensor(
                                Tacc[:], Tacc[:], g[:], OP.add
                            )

                    # attn = E * recip (broadcast over l), stream out
                    nc.vector.tensor_tensor(
                        at[:],
                        E[:],
                        rec[:, :, None].to_broadcast((P, A, L)),
                        OP.mult,
                    )
                    nc.sync.dma_start(attn_d[h], at[:])

                # weight = tanh(T1 + T2^T + b + 0.5) * (1 - eye)
                pst = psT.tile([P, A], f32, tag="pst", name="pst_t")
                nc.tensor.transpose(pst[:], T2[:], idn[:])
                logits = small.tile([P, A], f32, tag="logits")
                nc.vector.tensor_tensor(logits[:], T1[:], pst[:], OP.add)
                wt = small.tile([P, A], f32, tag="wt")
                nc.scalar.activation(
                    wt[:], logits[:], AF.Tanh, bias=bias_sb[:, :], scale=1.0
                )
                nc.vector.tensor_tensor(wt[:], wt[:], mask_sb[:], OP.mult)
                nc.sync.dma_start(weight_d[:], wt[:])

    nc.compile()
    return nc


def _get_nc():
    if "nc" not in _NC_CACHE:
        _NC_CACHE["nc"] = _build_nc()
    return _NC_CACHE["nc"]


def _prep_in_maps(output, Wk, Wq, Wv, Watt, b_att, agent_num):
    output = np.ascontiguousarray(np.asarray(output, dtype=np.float32))
    Wk = np.asarray(Wk, dtype=np.float32)
    Wq = np.asarray(Wq, dtype=np.float32)
    Wv = np.asarray(Wv, dtype=np.float32)
    Watt = np.asarray(Watt, dtype=np.float32).reshape(2 * D)
    b0 = float(np.asarray(b_att, dtype=np.float32).reshape(-1)[0])
    assert int(agent_num) == A

    # fold Wv with the two Watt halves: vw1[c,h], vw2[c,h]
    wv1 = np.stack(
        [Wv[:, h * HD : (h + 1) * HD] @ Watt[h * HD : (h + 1) * HD] for h in range(H)],
        axis=1,
    )
    wv2 = np.stack(
        [
            Wv[:, h * HD : (h + 1) * HD] @ Watt[D + h * HD : D + (h + 1) * HD]
            for h in range(H)
        ],
        axis=1,
    )
    wv12 = np.ascontiguousarray(
        np.concatenate([wv1, wv2], axis=1).astype(np.float32).reshape(2, P, 2 * H)
    )
    wk2 = np.ascontiguousarray(Wk.reshape(2, P, D))
    wq2 = np.ascontiguousarray(Wq.reshape(2, P, D))
    mask = np.ascontiguousarray((1.0 - np.eye(A)).astype(np.float32))
    biasv = np.full((P, 1), b0 + 0.5, dtype=np.float32)

    in_maps = []
    for b in range(B):
        xb = output[:, b * A : (b + 1) * A, :].reshape(N, D)
        xT = np.ascontiguousarray(xb.T).reshape(2, P, N)
        in_maps.append(
            {
                "xT": xT,
                "wk": wk2,
                "wq": wq2,
                "wv12": wv12,
                "mask": mask,
                "biasv": biasv,
            }
        )
    return in_maps


def kernel(output, Wk, Wq, Wv, Watt, b_att, agent_num):
    global LAST_RESULT
    from concourse.bass_utils import run_bass_kernel_spmd

    in_maps = _prep_in_maps(output, Wk, Wq, Wv, Watt, b_att, agent_num)
    nc = _get_nc()
    res = run_bass_kernel_spmd(nc, in_maps, core_ids=list(range(B)))
    LAST_RESULT = res

    attn = np.stack([res.results[b]["attn"] for b in range(B)], axis=0)
    weight = np.stack([res.results[b]["weight"] for b in range(B)], axis=0)
    return attn, weight


if __name__ == "__main__":
    rng = np.random.default_rng(0)
    inputs = {
        "output": rng.standard_normal((L, B * A, D), dtype=np.float32),
        "Wk": rng.standard_normal((D, D), dtype=np.float32) / 16,
        "Wq": rng.standard_normal((D, D), dtype=np.float32) / 16,
        "Wv": rng.standard_normal((D, D), dtype=np.float32) / 16,
        "Watt": rng.standard_normal((2 * D, 1), dtype=np.float32) / 16,
        "b_att": np.zeros(1, dtype=np.float32),
        "agent_num": A,
    }
    a, w = kernel(**inputs)
    print(a.shape, w.shape)


# revision 12
# speedup vs baseline: 1.0213x; 1.0213x over previous
"""Trainium2 Bass kernel for nn_Attention_88510686036059 (gnn message passing).

Strategy: data-parallel over batch B=8 -> one NeuronCore per batch element.
Per core (batch b):
  x_b (L=32, A=128, D=256) pre-transposed on host to xT (D, L*A).
  kT/qT = Wk^T/Wq^T projections via PE in f32r (W stationary), evicted bf16,
  layout (dout, l*A+a).
  scores per (h,l): S = kT_hl^T @ qT_hl on PE (K=hd=32, bf16), 8 l's per
  PSUM tile; one ACT exp per tile evicts into slot 2 of a 3-slot bf16 tile
  gt[i, j, slot, l] (slot0=E*vw1, slot1=E*vw2, slot2=E).
  One DVE tensor_reduce over the 3-slot tile yields [G1, G2, denom] at once.
  weight head (algebraic collapse -- out/gat never materialized):
    logits[i,j] = sum_h rec[h,i,j]*G1[h,i,j] + (sum_h rec*G2)^T[i,j] + b
    vw1/vw2 = x @ (Wv . Watt-half folding), computed on PE.
  attn = E * recip (bf16 2x on DVE), written out via gpsimd SWDGE with
  bf16->f32 dtype conversion; 2MB contiguous per (h).
"""

import numpy as np

L, B, A, D, H = 32, 8, 128, 256, 8
HD = D // H
P = 128
N = L * A  # 4096
LB = 8  # l's per score PSUM tile / exp op

LAST_RESULT = None
_NC_CACHE = {}


def _build_nc():
    import concourse.bass as bass  # noqa: F401
    import concourse.mybir as mybir
    import concourse.tile as tile
    from concourse import bacc
    from concourse.masks import make_identity

    f32 = mybir.dt.float32
    f32r = mybir.dt.float32r
    bf16 = mybir.dt.bfloat16
    AF = mybir.ActivationFunctionType
    OP = mybir.AluOpType
    AX = mybir.AxisListType

    nc = bacc.Bacc(
        "TRN2", target_bir_lowering=False, debug=False, num_devices=B
    )

    xT_d = nc.dram_tensor("xT", (2, P, N), f32r, kind="ExternalInput")
    wk_d = nc.dram_tensor("wk", (2, P, D), f32r, kind="ExternalInput")
    wq_d = nc.dram_tensor("wq", (2, P, D), f32r, kind="ExternalInput")
    wv_d = nc.dram_tensor("wv12", (2, P, 2 * H), f32r, kind="ExternalInput")
    mask_d = nc.dram_tensor("mask", (P, A), f32, kind="ExternalInput")
    bias_d = nc.dram_tensor("biasv", (P, 1), f32, kind="ExternalInput")
    attn_d = nc.dram_tensor("attn", (H, A, A, L), f32, kind="ExternalOutput")
    weight_d = nc.dram_tensor("weight", (A, A), f32, kind="ExternalOutput")

    with tile.TileContext(nc) as tc:
        with (
            tc.tile_pool(name="const", bufs=1) as cpool,
            tc.tile_pool(name="kqt", bufs=1) as kqt_pool,
            tc.tile_pool(name="small", bufs=1) as small,
            tc.tile_pool(name="smallh", bufs=2) as smallh,
        ):
            # persistent tiles
            kT = [kqt_pool.tile([P, N], bf16, tag=f"kT{m}", name=f"kT{m}")
                  for m in range(2)]
            qT = [kqt_pool.tile([P, N], bf16, tag=f"qT{m}", name=f"qT{m}")
                  for m in range(2)]
            vw_sb = small.tile([P, 2 * H, L], bf16, tag="vw")  # [a, g, l]
            idn = cpool.tile([P, P], f32, tag="idn")
            make_identity(nc, idn[:])
            mask_sb = cpool.tile([P, A], f32, tag="mask")
            bias_sb = cpool.tile([P, 1], f32, tag="bias")
            nc.sync.dma_start(mask_sb[:], mask_d[:])
            nc.sync.dma_start(bias_sb[:], bias_d[:])
            T12 = small.tile([P, A, 2], f32, tag="T12")

            # ---------------- phase 1: projections ----------------
            with (
                tc.tile_pool(name="xpool", bufs=1) as xpool,
                tc.tile_pool(name="psA", bufs=4, space="PSUM") as psA,
                tc.tile_pool(name="psB", bufs=2, space="PSUM") as psB,
            ):
                xT = [xpool.tile([P, N], f32r, tag=f"xT{c}", name=f"xT{c}")
                      for c in range(2)]
                for c in range(2):
                    nc.sync.dma_start(xT[c][:], xT_d[c])
                wk_sb = cpool.tile([P, 2, D], f32r, tag="wk")
                wq_sb = cpool.tile([P, 2, D], f32r, tag="wq")
                wv_sb = cpool.tile([P, 2, 2 * H], f32r, tag="wv")
                nc.sync.dma_start(wk_sb[:], wk_d[:].rearrange("c p d -> p c d"))
                nc.sync.dma_start(wq_sb[:], wq_d[:].rearrange("c p d -> p c d"))
                nc.sync.dma_start(wv_sb[:], wv_d[:].rearrange("c p d -> p c d"))

                # kT/qT = W^T x^T : lhsT = W chunk (K=din 128, M=dout 128)
                for wsb, dst, ev in ((wk_sb, kT, "v"), (wq_sb, qT, "s")):
                    for m in range(2):
                        for ns in range(8):
                            ps = psA.tile([P, 512], f32, tag="psA",
                                          name="psA_t")
                            for c in range(2):
                                nc.tensor.matmul(
                                    ps[:],
                                    wsb[:, c, m * P : (m + 1) * P],
                                    xT[c][:, ns * 512 : (ns + 1) * 512],
                                    start=(c == 0),
                                    stop=(c == 1),
                                )
                            dslice = dst[m][:, ns * 512 : (ns + 1) * 512]
                            nc.scalar.copy(dslice, ps[:])

                # vw[a, g, l] = sum_c xT[c, l*128+a] * wv12[c, g]
                for l in range(L):
                    ps = psB.tile([P, 2 * H], f32, tag="psB", name="psB_t")
                    for c in range(2):
                        nc.tensor.matmul(
                            ps[:],
                            xT[c][:, l * P : (l + 1) * P],
                            wv_sb[:, c, :],
                            start=(c == 0),
                            stop=(c == 1),
                        )
                    nc.vector.tensor_copy(vw_sb[:, :, l], ps[:])

            # ---------------- phase 2: scores/softmax/gat ----------------
            with (
                tc.tile_pool(name="gpool", bufs=3) as gpool,
                tc.tile_pool(name="apool", bufs=3) as apool,
                tc.tile_pool(name="psS", bufs=3, space="PSUM") as psS,
                tc.tile_pool(name="psT", bufs=1, space="PSUM") as psT,
            ):
                for h in range(H):
                    ch, rh = h // 4, (h % 4) * 32
                    # gt[i, j, slot, l]: slot0=E*vw1, slot1=E*vw2, slot2=E
                    gt = gpool.tile([P, A, 3, L], bf16, tag="gt", name="gt_t")
                    at = apool.tile([P, A, L], bf16, tag="at", name="at_t")
                    for lb in range(L // LB):
                        ps = psS.tile([P, LB * A], f32, tag="psS",
                                      name="psS_t")
                        for li in range(LB):
                            l = lb * LB + li
                            nc.tensor.matmul(
                                ps[:, li * P : (li + 1) * P],
                                kT[ch][rh : rh + 32, l * P : (l + 1) * P],
                                qT[ch][rh : rh + 32, l * P : (l + 1) * P],
                                start=True,
                                stop=True,
                                tile_position=(rh, 0),
                            )
                        # E = exp(S / sqrt(D)) for LB l's in one ACT op
                        nc.scalar.activation(
                            gt[:, :, 2, lb * LB : (lb + 1) * LB],
                            ps[:].rearrange("p (l j) -> p j l", j=P),
                            AF.Exp,
                            scale=1.0 / 16.0,
                        )
                    # denom first (short critical path into recip/normalize)
                    den = smallh.tile([P, A], f32, tag="den", name="den_t")
                    nc.vector.tensor_reduce(den[:], gt[:, :, 2, :], axis=AX.X,
                                            op=OP.add)
                    rec = smallh.tile([P, A], f32, tag="rec", name="rec_t")
                    nc.vector.reciprocal(rec[:], den[:])
                    rec_bf = smallh.tile([P, A], bf16, tag="recbf",
                                         name="recbf_t")
                    nc.scalar.copy(rec_bf[:], rec[:])

                    # G muls: slot0 on DVE (bf16 2x), slot1 on GpSimd
                    nc.vector.tensor_tensor(
                        gt[:, :, 0, :],
                        gt[:, :, 2, :],
                        vw_sb[:, h, None, :].to_broadcast((P, A, L)),
                        OP.mult,
                    )
                    nc.gpsimd.tensor_tensor(
                        gt[:, :, 1, :],
                        gt[:, :, 2, :],
                        vw_sb[:, H + h, None, :].to_broadcast((P, A, L)),
                        OP.mult,
                    )
                    # fused reduce of both G slots -> (128, A, 2) f32
                    g3 = smallh.tile([P, A, 2], f32, tag="g3", name="g3_t")
                    nc.vector.tensor_reduce(g3[:], gt[:, :, 0:2, :], axis=AX.X,
                                            op=OP.add)

                    # T12 += [G1, G2] * rec (both slots in one op pair)
                    g = smallh.tile([P, A, 2], f32, tag="gm", name="gm_t")
                    nc.vector.tensor_tensor(
                        g[:], g3[:], rec[:, :, None].to_broadcast((P, A, 2)),
                        OP.mult,
                    )
                    if h == 0:
                        nc.vector.tensor_copy(T12[:], g[:])
                    else:
                        nc.vector.tensor_tensor(T12[:], T12[:], g[:], OP.add)

                    # attn = E * recip; split l-range between DVE and GpSimd
                    LS = 8
                    nc.vector.tensor_tensor(
                        at[:, :, :LS],
                        gt[:, :, 2, :LS],
                        rec_bf[:, :, None].to_broadcast((P, A, LS)),
                        OP.mult,
                    )
                    nc.gpsimd.tensor_tensor(
                        at[:, :, LS:],
                        gt[:, :, 2, LS:],
                        rec_bf[:, :, None].to_broadcast((P, A, L - LS)),
                        OP.mult,
                    )
                    # SWDGE dma converts bf16 -> f32 on the way out
                    nc.gpsimd.dma_start(attn_d[h], at[:])

                # weight = tanh(T1 + T2^T + b + 0.5) * (1 - eye)
                pst = psT.tile([P, A], f32, tag="pst", name="pst_t")
                nc.tensor.transpose(pst[:], T12[:, :, 1], idn[:])
                logits = small.tile([P, A], f32, tag="logits")
                nc.vector.tensor_tensor(logits[:], T12[:, :, 0], pst[:], OP.add)
                wt = small.tile([P, A], f32, tag="wt")
                nc.scalar.activation(
                    wt[:], logits[:], AF.Tanh, bias=bias_sb[:, :], scale=1.0
                )
                nc.vector.tensor_tensor(wt[:], wt[:], mask_sb[:], OP.mult)
                nc.sync.dma_start(weight_d[:], wt[:])

    nc.compile()
    return nc


def _get_nc():
    if "nc" not in _NC_CACHE:
        _NC_CACHE["nc"] = _build_nc()
    return _NC_CACHE["nc"]


def _prep_in_maps(output, Wk, Wq, Wv, Watt, b_att, agent_num):
    output = np.ascontiguousarray(np.asarray(output, dtype=np.float32))
    Wk = np.asarray(Wk, dtype=np.float32)
    Wq = np.asarray(Wq, dtype=np.float32)
    Wv = np.asarray(Wv, dtype=np.float32)
    Watt = np.asarray(Watt, dtype=np.float32).reshape(2 * D)
    b0 = float(np.asarray(b_att, dtype=np.float32).reshape(-1)[0])
    assert int(agent_num) == A

    # fold Wv with the two Watt halves: vw1[c,h], vw2[c,h]
    wv1 = np.stack(
        [Wv[:, h * HD : (h + 1) * HD] @ Watt[h * HD : (h + 1) * HD]
         for h in range(H)],
        axis=1,
    )
    wv2 = np.stack(
        [Wv[:, h * HD : (h + 1) * HD] @ Watt[D + h * HD : D + (h + 1) * HD]
         for h in range(H)],
        axis=1,
    )
    wv12 = np.ascontiguousarray(
        np.concatenate([wv1, wv2], axis=1).astype(np.float32)
        .reshape(2, P, 2 * H)
    )
    wk2 = np.ascontiguousarray(Wk.reshape(2, P, D))
    wq2 = np.ascontiguousarray(Wq.reshape(2, P, D))
    mask = np.ascontiguousarray((1.0 - np.eye(A)).astype(np.float32))
    biasv = np.full((P, 1), b0 + 0.5, dtype=np.float32)

    in_maps = []
    for b in range(B):
        xb = output[:, b * A : (b + 1) * A, :].reshape(N, D)
        xT = np.ascontiguousarray(xb.T).reshape(2, P, N)
        in_maps.append(
            {
                "xT": xT,
                "wk": wk2,
                "wq": wq2,
                "wv12": wv12,
                "mask": mask,
                "biasv": biasv,
            }
        )
    return in_maps


def kernel(output, Wk, Wq, Wv, Watt, b_att, agent_num):
    global LAST_RESULT
    from concourse.bass_utils import run_bass_kernel_spmd

    in_maps = _prep_in_maps(output, Wk, Wq, Wv, Watt, b_att, agent_num)
    nc = _get_nc()
    res = run_bass_kernel_spmd(nc, in_maps, core_ids=list(range(B)))
    LAST_RESULT = res

    attn = np.stack([res.results[b]["attn"] for b in range(B)], axis=0)
    weight = np.stack([res.results[b]["weight"] for b in range(B)], axis=0)
    return attn, weight


if __name__ == "__main__":
    rng = np.random.default_rng(0)
    inputs = {
        "output": rng.standard_normal((L, B * A, D), dtype=np.float32),
        "Wk": rng.standard_normal((D, D), dtype=np.float32) / 16,
        "Wq": rng.standard_normal((D, D), dtype=np.float32) / 16,
        "Wv": rng.standard_normal((D, D), dtype=np.float32) / 16,
        "Watt": rng.standard_normal((2 * D, 1), dtype=np.float32) / 16,
        "b_att": np.zeros(1, dtype=np.float32),
        "agent_num": A,
    }
    a, w = kernel(**inputs)
    print(a.shape, w.shape)
